# revision 7
# baseline (speedup 1.0000x reference)
"""Trainium2 Bass kernel for a single-layer multi-head self-attention.

Math per batch element b (one NeuronCore each):
    z[h] = W[h] @ x_b + b[h]          (d=32, L=1024) per head h in 0..7
    S    = z[h].T @ z[h] / sqrt(d)    (1024, 1024), symmetric since Q=K=V
    out[h] = softmax(S, axis=-1) @ z[h]   -> laid out (d, L) per head

Device layout tricks:
  - z_all  (256, 1024): heads stacked along partitions (d-major), feeds the
    score matmuls (lhsT and rhs are both 32-partition slices of z_all).
  - ztld   (1024, 264): L-major transposed z with a ones column appended per
    head ([z | 1] -> 33 cols/head), computed directly by a second projection
    matmul (no on-device transposes). Serves as lhsT of the AV matmul, whose
    row 32 then yields the softmax row-sums for free.
  - E = exp(S/sqrt(d)) is kept unnormalized; symmetric E means row-chunks of
    E stored (l-chunk partitions, m free) serve directly as the AV rhs.
  - Normalization: reciprocal of row-sums, partition-broadcast via DMA,
    one tensor_tensor multiply.
"""

import math

import numpy as np

_B, _C, _P, _T = 8, 256, 16, 64
_H, _D = 8, 32
_L = _P * _T  # 1024
_NCORES = 8
_SCALE = 1.0 / math.sqrt(_D)

_CACHE = {}


def _build_nc():
    import concourse.bacc as bacc
    import concourse.mybir as mybir
    import concourse.tile as tile

    fp32 = mybir.dt.float32
    AF = mybir.ActivationFunctionType
    ALU = mybir.AluOpType

    nc = bacc.Bacc()

    x_d = nc.dram_tensor("x", [_C, _L], fp32, kind="ExternalInput")
    wt_d = nc.dram_tensor("wt", [_C, _H * _D], fp32, kind="ExternalInput")
    wtt_d = nc.dram_tensor("wtt", [_C, _H * 33], fp32, kind="ExternalInput")
    btt_d = nc.dram_tensor("btt", [1, _H * 33], fp32, kind="ExternalInput")
    bias_d = nc.dram_tensor("bias", [_H * _D, 1], fp32, kind="ExternalInput")
    out_d = nc.dram_tensor("out", [_H * _D, _L], fp32, kind="ExternalOutput")

    with tile.TileContext(nc) as tc:
        with (
            tc.tile_pool(name="consts", bufs=1) as consts,
            tc.tile_pool(name="xz", bufs=1) as xz,
            tc.tile_pool(name="epool", bufs=16) as epool,
            tc.tile_pool(name="small", bufs=3) as small,
            tc.tile_pool(name="psum_big", bufs=2, space="PSUM") as psum_big,
            tc.tile_pool(name="psum_med", bufs=2, space="PSUM") as psum_med,
            tc.tile_pool(name="psum_av", bufs=2, space="PSUM") as psum_av,
            tc.tile_pool(name="dram", bufs=2, space="DRAM") as dram,
        ):
            # ---- load inputs ----
            x_sb = []
            wt_sb = []
            wtt_sb = []
            bias_sb = []
            for k in range(2):
                xk = xz.tile([128, _L], fp32, name=f"x{k}")
                nc.sync.dma_start(xk, x_d[128 * k : 128 * (k + 1), :])
                x_sb.append(xk)
                wtk = consts.tile([128, _H * _D], fp32, name=f"wt{k}")
                nc.sync.dma_start(wtk, wt_d[128 * k : 128 * (k + 1), :])
                wt_sb.append(wtk)
                wttk = consts.tile([128, _H * 33], fp32, name=f"wtt{k}")
                nc.sync.dma_start(wttk, wtt_d[128 * k : 128 * (k + 1), :])
                wtt_sb.append(wttk)
                bk = consts.tile([128, 1], fp32, name=f"bias{k}")
                nc.sync.dma_start(bk, bias_d[128 * k : 128 * (k + 1), :])
                bias_sb.append(bk)
            btt_sb = consts.tile([1, _H * 33], fp32, name="btt")
            nc.sync.dma_start(btt_sb, btt_d[:, :])
            ones_sb = consts.tile([1, 128], fp32, name="ones")
            nc.vector.memset(ones_sb, 1.0)

            # ---- z_all (256, 1024): heads-stacked projection + bias ----
            z_sb = []
            for m in range(2):
                zp = psum_big.tile([128, _L], fp32, name=f"zp{m}", tag="sbig")
                for half in range(2):
                    for k in range(2):
                        nc.tensor.matmul(
                            zp[:, 512 * half : 512 * (half + 1)],
                            wt_sb[k][:, 128 * m : 128 * (m + 1)],
                            x_sb[k][:, 512 * half : 512 * (half + 1)],
                            start=(k == 0),
                            stop=(k == 1),
                        )
                zm = xz.tile([128, _L], fp32, name=f"z{m}")
                nc.vector.tensor_scalar_add(zm, zp, bias_sb[m])
                z_sb.append(zm)

            # ---- ztld (8 x (128, 264)): L-major z with bias and ones col ----
            zt_sb = []
            for i in range(8):
                ztp = psum_med.tile([128, _H * 33], fp32, name=f"ztp{i}", tag="ztp")
                for k in range(2):
                    nc.tensor.matmul(
                        ztp,
                        x_sb[k][:, 128 * i : 128 * (i + 1)],
                        wtt_sb[k],
                        start=(k == 0),
                        stop=False,
                        skip_group_check=True,
                    )
                nc.tensor.matmul(
                    ztp,
                    ones_sb,
                    btt_sb,
                    start=False,
                    stop=True,
                    skip_group_check=True,
                )
                zti = xz.tile([128, _H * 33], fp32, name=f"zt{i}")
                nc.vector.tensor_copy(zti, ztp)
                zt_sb.append(zti)

            # ---- attention per head ----
            for h in range(8):
                m, s = h // 4, h % 4
                zh = z_sb[m][32 * s : 32 * (s + 1), :]  # (32, 1024)

                es = []
                for i in range(8):
                    sp = psum_big.tile([128, _L], fp32, name=f"sp{h}_{i}", tag="sbig")
                    for half in range(2):
                        nc.tensor.matmul(
                            sp[:, 512 * half : 512 * (half + 1)],
                            zh[:, 128 * i : 128 * (i + 1)],
                            zh[:, 512 * half : 512 * (half + 1)],
                            start=True,
                            stop=True,
                            tile_position=(32 * s, 0),
                        )
                    ei = epool.tile([128, _L], fp32, name=f"e{h}_{i}", tag="e")
                    nc.scalar.activation(ei, sp, AF.Exp, scale=_SCALE)
                    es.append(ei)

                # O.T (33, 1024) = ztld[:, head cols].T @ E  (row 32 = rowsums)
                avp = [
                    psum_av.tile([33, 512], fp32, name=f"av{h}_{half}", tag="av")
                    for half in range(2)
                ]
                for j in range(8):
                    lhsT = zt_sb[j][:, 33 * h : 33 * (h + 1)]
                    for half in range(2):
                        nc.tensor.matmul(
                            avp[half],
                            lhsT,
                            es[j][:, 512 * half : 512 * (half + 1)],
                            start=(j == 0),
                            stop=(j == 7),
                        )

                r = small.tile([1, _L], fp32, name=f"r{h}", tag="r")
                for half in range(2):
                    nc.vector.reciprocal(
                        r[:, 512 * half : 512 * (half + 1)], avp[half][32:33, :]
                    )
                rd = dram.tile([1, _L], fp32, name=f"rd{h}", tag="rd")
                nc.sync.dma_start(rd, r)
                rb = small.tile([32, _L], fp32, name=f"rb{h}", tag="rb")
                nc.sync.dma_start(rb, rd.to_broadcast([32, _L]))
                o = small.tile([32, _L], fp32, name=f"o{h}", tag="o")
                for half in range(2):
                    nc.vector.tensor_tensor(
                        o[:, 512 * half : 512 * (half + 1)],
                        avp[half][0:32, :],
                        rb[:, 512 * half : 512 * (half + 1)],
                        op=ALU.mult,
                    )
                nc.sync.dma_start(out_d[32 * h : 32 * (h + 1), :], o)

    nc.finalize()
    return nc


def _get_compiled():
    if "nc" not in _CACHE:
        _CACHE["nc"] = _build_nc()
    return _CACHE["nc"]


def kernel(x: np.ndarray, W: np.ndarray, b: np.ndarray) -> np.ndarray:
    from concourse.bass_utils import run_bass_kernel_spmd

    x = np.ascontiguousarray(x, dtype=np.float32)
    W = np.ascontiguousarray(W, dtype=np.float32)
    b = np.ascontiguousarray(b, dtype=np.float32)

    wt = np.ascontiguousarray(W.reshape(_H * _D, _C).T)  # (C, H*D)
    wtt = np.zeros((_C, _H * 33), dtype=np.float32)
    btt = np.zeros((1, _H * 33), dtype=np.float32)
    for h in range(_H):
        wtt[:, 33 * h : 33 * h + 32] = W[h].T
        btt[0, 33 * h : 33 * h + 32] = b[h]
        btt[0, 33 * h + 32] = 1.0
    bias = np.ascontiguousarray(b.reshape(_H * _D, 1))

    in_maps = [
        {
            "x": np.ascontiguousarray(x[i].reshape(_C, _L)),
            "wt": wt,
            "wtt": wtt,
            "btt": btt,
            "bias": bias,
        }
        for i in range(_NCORES)
    ]

    nc = _get_compiled()
    res = run_bass_kernel_spmd(nc, in_maps, list(range(_NCORES)))
    out = np.stack(
        [res.results[i]["out"].reshape(_H * _D, _P, _T) for i in range(_NCORES)]
    )
    return out
